# revision 39
# baseline (speedup 1.0000x reference)
"""BertAttention (QKV proj + MHA + out-proj + residual + LayerNorm) on 8
Trainium2 NeuronCores.

Sharding: tensor-parallel over heads. Core c owns heads {2c, 2c+1} (a
128-wide slice of the hidden dim): it computes Q/K/V projections for its
slice over the full batch*seq, runs attention for its 8 (batch, head)
pairs entirely out of SBUF. A per-batch AllToAll re-shards the attention
context from head-split to row-split: core c owns rows
[b*2048 + c*256, b*2048 + (c+1)*256) of EVERY batch b, so the output
projection + residual + LayerNorm for batch b overlaps the attention
compute of batch b+1 (instead of one serial tail after a single A2A).
Host reassembles the 8 cores' [B, 256, H] row shards.

Matmuls run in bf16 (fp32 PSUM accumulate); softmax and LayerNorm
arithmetic stay fp32.

softmax: scores are built transposed (scoresT[k, q] per head) so the
probs@V contraction needs no transpose; the row-sum comes from an extra
all-ones column appended to V; the attention mask enters as the
per-partition bias of the Exp activation (exp(s/8 + mask_k), mask only
depends on the key position k which is the partition axis of scoresT).

LayerNorm avoids ACT-table switches (Exp must stay resident for the
interleaved attention): mean/var via bn_stats/bn_aggr on the vector
engine, rstd via a Quake-style bit-trick + 2 Newton iterations on the
vector engine (the hardware has no rsqrt outside the ACT tables).

Scheduling notes (the bulk of the speedup over the first working
version): one big host-preswizzled DMA descriptor per tensor (the sync
engine pays ~0.7us per trigger); compute-dependent DMA triggers (A2A
staging, output stores) issue from the gpsimd queue so they never block
the sync queue's input stream; a tiny memset "fence" on the ctxF tile
keeps the collective-dependent out-proj matmuls from being statically
scheduled ahead of ready attention work; deep hsb buffering (bufs=7)
decouples next-batch input loads from the V-projection JIT scheduling;
batch 0 interleaves attention qg0 into the projection blocks; throwaway
matmuls warm the PE clock (HAM) through the startup DMA wait and the
final collective.
"""

import os
import sys
import contextlib
import ctypes
import types

import numpy as np
import ml_dtypes

N_CORES = 8
B, S, H = 4, 2048, 1024
NH, DH = 16, 64
R = B * S            # 8192 flattened rows
HB = H // N_CORES    # 128 head-dim columns per core (2 heads)
RPB = S // N_CORES   # 256 output rows per core per batch
SBW = 512            # seq-block width for projections
NSB = R // SBW       # 16 seq blocks
NHC = H // 128       # 8 contraction chunks of 128
NKB = S // 128       # 16 key blocks per batch
LN_EPS = 1e-12

last_exec_time_ns = None

# ---------------------------------------------------------------------------
# NTFF profile hook shim (axon images without antenv.axon_hooks).
# Only needed when tracing; harmless otherwise.
_SO_PATH = "/opt/axon/libaxon_pjrt.so"


def _install_ntff_shim():
    try:
        from antenv import axon_hooks  # noqa: F401
        return
    except ImportError:
        pass
    hook = None
    try:
        lib = ctypes.CDLL(_SO_PATH)
        if hasattr(lib, "axon_start_nrt_profile"):
            lib.axon_start_nrt_profile.argtypes = [
                ctypes.POINTER(ctypes.c_int64), ctypes.c_size_t]
            lib.axon_start_nrt_profile.restype = ctypes.c_int64
            lib.axon_stop_nrt_profile.argtypes = [ctypes.c_char_p]
            lib.axon_stop_nrt_profile.restype = ctypes.c_int64

            @contextlib.contextmanager
            def _hook(output_dir, device_ids):
                import jax
                jax.devices()
                if device_ids:
                    ids = (ctypes.c_int64 * len(device_ids))(*device_ids)
                    rc = lib.axon_start_nrt_profile(ids, len(device_ids))
                else:
                    rc = lib.axon_start_nrt_profile(None, 0)
                if rc != 0:
                    raise RuntimeError(f"axon_start_nrt_profile rc={rc}")
                try:
                    yield
                finally:
                    n = lib.axon_stop_nrt_profile(str(output_dir).encode())
                    print(f"profile: {n} ntff file(s) in {output_dir}",
                          file=sys.stderr)

            hook = _hook
    except OSError:
        pass
    mod = types.ModuleType("antenv.axon_hooks")
    mod._hook = hook
    mod.get_axon_ntff_profile_hook = lambda: mod._hook
    mod.set_axon_ntff_profile_hook = lambda h: setattr(mod, "_hook", h)
    sys.modules["antenv.axon_hooks"] = mod
    try:
        import antenv
        antenv.axon_hooks = mod
    except ImportError:
        pass


# ---------------------------------------------------------------------------

def _build():
    from concourse import bacc, tile
    import concourse.mybir as mybir

    f32 = mybir.dt.float32
    bf16 = mybir.dt.bfloat16
    AF = mybir.ActivationFunctionType
    ALU = mybir.AluOpType

    nc = bacc.Bacc("TRN2", target_bir_lowering=False, debug=False,
                   num_devices=N_CORES)

    # ---- DRAM I/O ----
    # Multi-chunk tensors are pre-swizzled on the host so each SBUF load is
    # ONE big DMA descriptor (the sync engine pays ~0.7us per trigger).
    hT_d = nc.dram_tensor("hT", [128, NSB, NHC, SBW], bf16,
                          kind="ExternalInput")
    wqT_d = nc.dram_tensor("wqT", [128, NHC, HB], bf16, kind="ExternalInput")
    wkT_d = nc.dram_tensor("wkT", [128, NHC, HB], bf16, kind="ExternalInput")
    wvT_d = nc.dram_tensor("wvT", [128, NHC, HB], bf16, kind="ExternalInput")
    woT_d = nc.dram_tensor("woT", [128, NHC, H], bf16, kind="ExternalInput")
    bq_d = nc.dram_tensor("bq", [HB], f32, kind="ExternalInput")
    bk_d = nc.dram_tensor("bk", [HB], f32, kind="ExternalInput")
    bv_d = nc.dram_tensor("bv", [HB], f32, kind="ExternalInput")
    bo_d = nc.dram_tensor("bo", [H], f32, kind="ExternalInput")
    gamma_d = nc.dram_tensor("gamma", [H], f32, kind="ExternalInput")
    beta_d = nc.dram_tensor("beta", [H], f32, kind="ExternalInput")
    maskT_d = nc.dram_tensor("maskT", [128, NKB, B], f32,
                             kind="ExternalInput")
    hres_d = nc.dram_tensor("hres", [B, 128, RPB // 128, H], f32,
                            kind="ExternalInput")
    out_d = nc.dram_tensor("out", [B, RPB, H], f32, kind="ExternalOutput")

    with tile.TileContext(nc) as tc:
        with (
            tc.tile_pool(name="const", bufs=1) as cpool,
            tc.tile_pool(name="psA", bufs=2, space="PSUM") as psA,
            tc.tile_pool(name="psB", bufs=2, space="PSUM") as psB,
            tc.tile_pool(name="psC", bufs=2, space="PSUM") as psC,
            tc.tile_pool(name="dram", bufs=1, space="DRAM") as dpool,
            tc.tile_pool(name="attn", bufs=2) as apool,
            tc.tile_pool(name="ptp", bufs=5) as ptpool,
            tc.tile_pool(name="bp", bufs=2) as bpool,
            tc.tile_pool(name="outp", bufs=2) as opool,
        ):
            # ================= setup (projection-phase constants) =========
            # K/Q weights first (K-projections lead), then the first hsb
            # blocks; the small bias/mask/wv loads are emitted after
            # emit_proj(0) so they queue behind the first hidden blocks.
            wq_sb = cpool.tile([128, NHC, HB], bf16, tag="wq")
            wk_sb = cpool.tile([128, NHC, HB], bf16, tag="wk")
            wv_sb = cpool.tile([128, NHC, HB], bf16, tag="wv")
            nc.sync.dma_start(wk_sb[:, :, :], wkT_d[:, :, :])
            nc.sync.dma_start(wq_sb[:, :, :], wqT_d[:, :, :])

            # PE warm-up: throwaway matmuls on the K weights keep the
            # HAM activity window busy while the first hidden blocks stream
            # in, so the first projections run at 2.4 GHz instead of 1.2.
            for w in range(16):
                pwarm = psA.tile([128, SBW], f32, tag="proj")
                nc.tensor.matmul(pwarm[:, :], wk_sb[:, w % NHC, :],
                                 wk_sb[:, 0:4, :], start=True, stop=True)

            bq_sb = cpool.tile([128, 1], f32, tag="bq")
            bk_sb = cpool.tile([128, 1], f32, tag="bk")
            bv_b = cpool.tile([128, HB], f32, tag="bv_b")
            mask_sb = cpool.tile([128, NKB, B], f32, tag="mask")

            def emit_small_loads():
                nc.sync.dma_start(wv_sb[:, :, :], wvT_d[:, :, :])
                nc.sync.dma_start(bk_sb[:, :], bk_d[:].unsqueeze(1))
                nc.sync.dma_start(bq_sb[:, :], bq_d[:].unsqueeze(1))
                # bv broadcast along partitions (V is in [seq, d] layout)
                nc.sync.dma_start(
                    bv_b[:, :],
                    bv_d[:].unsqueeze(0).partition_broadcast(128))
                # raw mask [k-in-block, kblock, batch]; consumed as the
                # per-partition bias of the Exp activation.
                nc.sync.dma_start(mask_sb[:, :, :], maskT_d[:, :, :])

            # A2A staging buffers (DRAM), one pair per batch
            a2a_in = [dpool.tile([N_CORES, 128, RPB], bf16, tag=f"a2a_in{b}",
                                 name=f"a2a_in{b}")
                      for b in range(B)]
            a2a_out = [dpool.tile([N_CORES, 128, RPB], bf16,
                                  tag=f"a2a_out{b}", name=f"a2a_out{b}")
                       for b in range(B)]

            # out-proj phase constants: declared here, loaded (by emission
            # order => scheduler priority) after batch 0 is enqueued so the
            # big wo/hres DMAs don't delay the first hidden-state blocks.
            wo_sb = cpool.tile([128, NHC, H], bf16, tag="wo")
            bo_b = cpool.tile([128, H], f32, tag="bo_b")
            gamma_b = cpool.tile([128, H], f32, tag="gamma_b")
            beta_b = cpool.tile([128, H], f32, tag="beta_b")


            def emit_kb(b, qg, kb, qt_sb, kt_sb, va, pc0, pc1):
                """One kb step of attention: scores pair (row-tile packed),
                exp (mask as ACT bias), probs@V accumulate."""
                sblk = kb // 4
                kcol = 128 * (kb % 4)
                sc = psB.tile([128, 2 * SBW], f32, tag="sc")
                nc.tensor.matmul(
                    sc[:, 0:SBW],
                    kt_sb[0:DH, sblk, kcol:kcol + 128],
                    qt_sb[0:DH, qg, :],
                    start=True, stop=True)
                nc.tensor.matmul(
                    sc[:, SBW:2 * SBW],
                    kt_sb[DH:2 * DH, sblk, kcol:kcol + 128],
                    qt_sb[DH:2 * DH, qg, :],
                    start=True, stop=True)
                pt = ptpool.tile([128, 2 * SBW], bf16, tag="pt")
                nc.scalar.activation(pt[:, :], sc[:, :], AF.Exp,
                                     bias=mask_sb[:, kb, b].unsqueeze(1),
                                     scale=0.125)
                nc.tensor.matmul(pc0[:, :], va[0][:, kb, :],
                                 pt[:, 0:SBW],
                                 start=(kb == 0), stop=(kb == NKB - 1))
                nc.tensor.matmul(pc1[:, :], va[1][:, kb, :],
                                 pt[:, SBW:2 * SBW],
                                 start=(kb == 0), stop=(kb == NKB - 1))

            def emit_normalize(b, qg, ctxT_b, pc0, pc1):
                """ctxT[d, q] = ctx'[d, q] / rowsum[q]. Copies release the
                psC slots first; reciprocal runs broadcast (64 lanes)."""
                cu = apool.tile([DH, 2, SBW], f32, tag="cu")
                rs = apool.tile([1, 2, SBW], f32, tag="rs")
                nc.vector.tensor_copy(cu[:, 0, :], pc0[0:DH, :])
                nc.vector.tensor_copy(rs[:, 0, :], pc0[DH:DH + 1, :])
                nc.vector.tensor_copy(cu[:, 1, :], pc1[0:DH, :])
                nc.vector.tensor_copy(rs[:, 1, :], pc1[DH:DH + 1, :])
                rb = [apool.tile([DH, SBW], f32, tag=f"rb{h}",
                                 name=f"rb{h}") for h in range(2)]
                for h in range(2):
                    nc.gpsimd.partition_broadcast(rb[h][:, :], rs[:, h, :])
                    nc.vector.reciprocal_approx_fast(rb[h][:, :],
                                                     rb[h][:, :])
                nc.vector.tensor_mul(ctxT_b[0:DH, qg, :],
                                     cu[0:DH, 0, :], rb[0][:, :])
                nc.vector.tensor_mul(ctxT_b[DH:2 * DH, qg, :],
                                     cu[0:DH, 1, :], rb[1][:, :])

            def emit_batch(b, va, qt_sb, kt_sb, ctxT_b, mid_emit):
                """Projections for batch b interleaved with attention qg0
                (qg0's kb 4j..4j+3 only need key/value block j), then Q for
                blocks 1-3, then attention qg1-3. For b>0 the projections
                get a priority offset so the static schedule interleaves
                them into attention(b-1)'s PE gaps instead of serializing
                them at the batch boundary."""
                import contextlib
                hsbs = []
                for i in range(4 * b, 4 * b + 4):
                    hsb = apool.tile([128, NHC, SBW], bf16, tag="hsb",
                                     bufs=8)
                    nc.sync.dma_start(hsb[:, :, :], hT_d[:, i, :, :])
                    hsbs.append(hsb)
                if b == 0:
                    emit_small_loads()
                pc0_0 = psC.tile([DH + 1, SBW], f32, tag="ctx")
                pc1_0 = psC.tile([DH + 1, SBW], f32, tag="ctx")
                for j in range(4):
                    i = 4 * b + j
                    hsb = hsbs[j]
                    # K^T block
                    pk = psA.tile([128, SBW], f32, tag="proj")
                    for c in range(NHC):
                        nc.tensor.matmul(pk[:, :], wk_sb[:, c, :],
                                         hsb[:, c, :],
                                         start=(c == 0),
                                         stop=(c == NHC - 1))
                    nc.vector.tensor_scalar_add(kt_sb[:, j, :],
                                                pk[:, :], bk_sb[:, :])
                    if j == 0 or b > 0:
                        # Q^T block
                        pq = psA.tile([128, SBW], f32, tag="proj")
                        for c in range(NHC):
                            nc.tensor.matmul(pq[:, :], wq_sb[:, c, :],
                                             hsb[:, c, :],
                                             start=(c == 0),
                                             stop=(c == NHC - 1))
                        nc.vector.tensor_scalar_add(qt_sb[:, j, :],
                                                    pq[:, :], bq_sb[:, :])
                    # V in natural [seq, d] layout, 4 sub-blocks of 128
                    for sub in range(4):
                        kb = 4 * j + sub
                        pv = psA.tile([128, SBW], f32, tag="proj")
                        for c in range(NHC):
                            nc.tensor.matmul(
                                pv[:, 0:HB],
                                hsb[:, c, 128 * sub:128 * (sub + 1)],
                                wv_sb[:, c, :],
                                start=(c == 0), stop=(c == NHC - 1))
                        nc.vector.tensor_add(va[0][:, kb, 0:DH],
                                             pv[:, 0:DH], bv_b[:, 0:DH])
                        nc.vector.tensor_add(va[1][:, kb, 0:DH],
                                             pv[:, DH:HB], bv_b[:, DH:HB])
                    if b == 0:
                        # batch 0 only: start qg0 right behind each block
                        # (for b>0 this would park proj(b) behind an exp-
                        # gated probs@V in the PE FIFO during attn(b-1)).
                        for kb in range(4 * j, 4 * j + 4):
                            emit_kb(b, 0, kb, qt_sb, kt_sb, va,
                                    pc0_0, pc1_0)
                if b == 0:
                    # remaining Q blocks
                    for j in range(1, 4):
                        hsb = hsbs[j]
                        pq = psA.tile([128, SBW], f32, tag="proj")
                        for c in range(NHC):
                            nc.tensor.matmul(pq[:, :], wq_sb[:, c, :],
                                             hsb[:, c, :],
                                             start=(c == 0),
                                             stop=(c == NHC - 1))
                        nc.vector.tensor_scalar_add(qt_sb[:, j, :],
                                                    pq[:, :], bq_sb[:, :])
                else:
                    for kb in range(NKB):
                        emit_kb(b, 0, kb, qt_sb, kt_sb, va, pc0_0, pc1_0)
                emit_normalize(b, 0, ctxT_b, pc0_0, pc1_0)
                if mid_emit is not None:
                    mid_emit()
                for qg in range(1, 4):
                    pc0 = psC.tile([DH + 1, SBW], f32, tag="ctx")
                    pc1 = psC.tile([DH + 1, SBW], f32, tag="ctx")
                    for kb in range(NKB):
                        emit_kb(b, qg, kb, qt_sb, kt_sb, va, pc0, pc1)
                    emit_normalize(b, qg, ctxT_b, pc0, pc1)

            i32 = mybir.dt.int32
            # Schraudolph/Quake constant: 1.5 * (127 - 0.0450466) * 2^23
            QK_C = 1.5 * (127.0 - 0.0450466) * 8388608.0

            def emit_outproj(b, ctxFb, hres_b):
                """Out-proj + residual + LayerNorm for this core's 256 rows
                of batch b. No ACT-table switches: LN runs on DVE only,
                rstd via bit-trick + 2 Newton iterations (rel err ~5e-6)."""
                nt = RPB // 128
                for t in range(nt):
                    nc.vector.tensor_add(hres_b[:, t, :], hres_b[:, t, :],
                                         bo_b[:, :])
                mv_b = opool.tile([128, nt, 2], f32, tag="bnmv")
                xs = []
                for t in range(nt):
                    x_sb = opool.tile([128, H], f32, tag=f"xln{t}",
                                      name=f"xln{t}", bufs=1)
                    for g in range(2):
                        po = psA.tile([128, SBW], f32, tag="proj")
                        for c in range(NHC):
                            nc.tensor.matmul(
                                po[:, :],
                                ctxFb[:, c, 128 * t:128 * (t + 1)],
                                wo_sb[:, c, SBW * g:SBW * (g + 1)],
                                start=(c == 0), stop=(c == NHC - 1))
                        nc.vector.tensor_add(
                            x_sb[:, SBW * g:SBW * (g + 1)], po[:, :],
                            hres_b[:, t, SBW * g:SBW * (g + 1)])
                    stats = opool.tile([128, 2, 6], f32, tag="bnst")
                    nc.vector.bn_stats(stats[:, 0, :], x_sb[:, 0:SBW])
                    nc.vector.bn_stats(stats[:, 1, :], x_sb[:, SBW:2 * SBW])
                    nc.vector.bn_aggr(mv_b[:, t, :], stats[:, :, :])
                    xs.append(x_sb)
                # rstd = (var + eps)^-0.5 for all row-tiles at once [128, nt]
                vv = opool.tile([128, nt], f32, tag="q_vv")
                nc.vector.tensor_scalar_add(vv[:, :], mv_b[:, :, 1], LN_EPS)
                qf = opool.tile([128, nt], f32, tag="q_f")
                nc.vector.tensor_copy(qf[:, :], vv[:, :].bitcast(i32))
                nc.vector.tensor_scalar(qf[:, :], qf[:, :], -0.5, QK_C,
                                        ALU.mult, ALU.add)
                qi = opool.tile([128, nt], i32, tag="q_i")
                nc.vector.tensor_copy(qi[:, :], qf[:, :])
                y0 = qi[:, :].bitcast(f32)
                rstd = opool.tile([128, nt], f32, tag="rstd")
                tn = opool.tile([128, nt], f32, tag="q_tn")
                for it in range(2):
                    src = y0 if it == 0 else rstd[:, :]
                    nc.vector.tensor_mul(tn[:, :], vv[:, :], src)
                    nc.vector.tensor_mul(tn[:, :], tn[:, :], src)
                    nc.vector.tensor_scalar(tn[:, :], tn[:, :], -0.5, 1.5,
                                            ALU.mult, ALU.add)
                    nc.vector.tensor_mul(rstd[:, :], src, tn[:, :])
                for t in range(nt):
                    # y = ((x - mu) * gamma) * rstd + beta
                    y_sb = opool.tile([128, H], f32, tag="yln")
                    nc.vector.scalar_tensor_tensor(y_sb[:, :], xs[t][:, :],
                                                   mv_b[:, t, 0:1],
                                                   gamma_b[:, :],
                                                   ALU.subtract, ALU.mult)
                    nc.vector.tensor_scalar_mul(y_sb[:, :], y_sb[:, :],
                                                rstd[:, t:t + 1])
                    nc.vector.tensor_add(y_sb[:, :], y_sb[:, :],
                                         beta_b[:, :])
                    nc.gpsimd.dma_start(out_d[b, 128 * t:128 * (t + 1), :],
                                        y_sb[:, :])

            # ================= main pipeline ==============================
            hres_tiles = {}
            for b in range(B):
                va = [apool.tile([128, NKB, DH + 1], bf16, tag=f"va{h}",
                                 name=f"va{h}") for h in range(2)]
                for h in range(2):
                    nc.vector.memset(va[h][:, :, DH:DH + 1], 1.0)
                qt_sb = apool.tile([128, 4, SBW], bf16, tag="qt")
                kt_sb = apool.tile([128, 4, SBW], bf16, tag="kt")
                ctxT_b = bpool.tile([128, 4, SBW], bf16, tag="ctxT")

                def mid_emit(b=b):
                    """Emitted right after qg0(b)'s normalize: out-proj of
                    batch b-1. The tiny memset is a scheduling fence -- it
                    makes the cc-dependent out-proj matmuls depend on
                    in-pipeline work, so the static PE order can't park them
                    (and everything behind them) on the collective."""
                    if b == 0:
                        return
                    ctxFb = opool.tile([128, N_CORES, RPB], bf16,
                                       tag="ctxF", name="ctxFb")
                    nc.vector.memset(ctxFb[0:1, 0:1], 0.0)
                    for src_ in range(N_CORES):
                        nc.sync.dma_start(ctxFb[:, src_, :],
                                          a2a_out[b - 1][src_, :, :])
                    emit_outproj(b - 1, ctxFb, hres_tiles.pop(b - 1))

                emit_batch(b, va, qt_sb, kt_sb, ctxT_b, mid_emit)
                # stage this batch's 8 row-chunks and run its AllToAll.
                # Triggered from the gpsimd queue (not sync, where it would
                # block the input-stream DMAs of the next batch).
                for j in range(N_CORES):
                    eng = nc.scalar if (b == B - 1 and j % 2 == 1) else nc.gpsimd
                    eng.dma_start(
                        a2a_in[b][j, :, :],
                        ctxT_b[:, j // 2, RPB * (j % 2):RPB * (j % 2 + 1)])
                nc.gpsimd.collective_compute(
                    "AllToAll", ALU.bypass,
                    replica_groups=[list(range(N_CORES))],
                    ins=[a2a_in[b][:].opt()], outs=[a2a_out[b][:].opt()])

                if b == 0:
                    # deprioritized big loads: needed first by outproj(0),
                    # which runs during batch 1's attention.
                    nc.sync.dma_start(wo_sb[:, :, :], woT_d[:, :, :])
                    nc.sync.dma_start(
                        bo_b[:, :],
                        bo_d[:].unsqueeze(0).partition_broadcast(128))
                    nc.sync.dma_start(
                        gamma_b[:, :],
                        gamma_d[:].unsqueeze(0).partition_broadcast(128))
                    nc.sync.dma_start(
                        beta_b[:, :],
                        beta_d[:].unsqueeze(0).partition_broadcast(128))

                # residual rows for batch b, consumed by outproj(b)
                hres_b = opool.tile([128, RPB // 128, H], f32, tag="hres")
                nc.sync.dma_start(hres_b[:, :, :], hres_d[b, :, :, :])
                hres_tiles[b] = hres_b

            # keep the PE's HAM activity window busy through the last
            # collective so outproj(3) runs at full clock, not 1.2 GHz
            for w in range(48):
                pwarm = psA.tile([128, SBW], f32, tag="proj")
                nc.tensor.matmul(pwarm[:, :], wo_sb[:, w % NHC, 0:128],
                                 wo_sb[:, w % NHC, 0:SBW],
                                 start=True, stop=True)
            ctxFb = opool.tile([128, N_CORES, RPB], bf16, tag="ctxF",
                               name="ctxFb")
            for src_ in range(N_CORES):
                eng = nc.scalar if src_ % 2 == 1 else nc.sync
                eng.dma_start(ctxFb[:, src_, :],
                              a2a_out[B - 1][src_, :, :])
            emit_outproj(B - 1, ctxFb, hres_tiles.pop(B - 1))

    nc.compile()
    return nc


_NC_CACHE = None


def _get_nc():
    global _NC_CACHE
    if _NC_CACHE is None:
        _NC_CACHE = _build()
    return _NC_CACHE


def _swz(mat_T):
    """[128*C, F] -> [128, C, F] with partition dim first (one-DMA layout)."""
    n = mat_T.shape[0] // 128
    return np.ascontiguousarray(
        mat_T.reshape(n, 128, -1).transpose(1, 0, 2))


def _make_in_maps(hidden_states, attention_mask, Wq, bq, Wk, bk, Wv, bv, Wo,
                  bo, ln_gamma, ln_beta):
    hid2 = np.asarray(hidden_states, np.float32).reshape(R, H)
    hT_bf = hid2.T.astype(ml_dtypes.bfloat16)          # [H, R]
    # [128, NSB, NHC, SBW]: hT3[p, i, c, q] = hT[128c+p, 512i+q]
    hT3 = np.ascontiguousarray(
        hT_bf.reshape(NHC, 128, NSB, SBW).transpose(1, 2, 0, 3))
    woT = _swz(np.asarray(Wo, np.float32).T.astype(ml_dtypes.bfloat16))
    maskT = _swz(np.ascontiguousarray(
        np.asarray(attention_mask, np.float32).reshape(B, S).T))
    bo32 = np.asarray(bo, np.float32)
    gamma32 = np.asarray(ln_gamma, np.float32)
    beta32 = np.asarray(ln_beta, np.float32)
    # [B, 128, RPB//128, H]: hres[b, p, t, :] = row b*S + c*RPB + 128t + p
    hid5 = hid2.reshape(B, N_CORES, RPB // 128, 128, H).transpose(0, 1, 3, 2, 4)

    in_maps = []
    for c in range(N_CORES):
        sl = slice(HB * c, HB * (c + 1))
        in_maps.append({
            "hT": hT3,
            "wqT": _swz(np.asarray(Wq, np.float32)[sl, :].T
                        .astype(ml_dtypes.bfloat16)),
            "wkT": _swz(np.asarray(Wk, np.float32)[sl, :].T
                        .astype(ml_dtypes.bfloat16)),
            "wvT": _swz(np.asarray(Wv, np.float32)[sl, :].T
                        .astype(ml_dtypes.bfloat16)),
            "woT": woT,
            "bq": np.ascontiguousarray(np.asarray(bq, np.float32)[sl]),
            "bk": np.ascontiguousarray(np.asarray(bk, np.float32)[sl]),
            "bv": np.ascontiguousarray(np.asarray(bv, np.float32)[sl]),
            "bo": bo32,
            "gamma": gamma32,
            "beta": beta32,
            "maskT": maskT,
            "hres": np.ascontiguousarray(hid5[:, c]),
        })
    return in_maps


def kernel(hidden_states, attention_mask, Wq, bq, Wk, bk, Wv, bv, Wo, bo,
           ln_gamma, ln_beta):
    global last_exec_time_ns
    from concourse.bass_utils import run_bass_kernel_spmd

    _install_ntff_shim()
    in_maps = _make_in_maps(hidden_states, attention_mask, Wq, bq, Wk, bk,
                            Wv, bv, Wo, bo, ln_gamma, ln_beta)
    nc = _get_nc()
    trace = os.environ.get("BASS_KERNEL_TRACE", "0") == "1"
    res = run_bass_kernel_spmd(nc, in_maps, core_ids=list(range(N_CORES)),
                               trace=trace)
    last_exec_time_ns = res.exec_time_ns
    if trace and res.exec_time_ns is not None:
        print(f"HW exec time: {res.exec_time_ns} ns")

    # res[c]["out"] is [B, RPB, H]; rows of batch b on core c are
    # [b*S + c*RPB, b*S + (c+1)*RPB).
    shards = np.stack([res.results[c]["out"] for c in range(N_CORES)],
                      axis=1)          # [B, N_CORES, RPB, H]
    return shards.reshape(B, S, H).astype(np.float32)


# revision 40
# speedup vs baseline: 1.0161x; 1.0161x over previous
"""BertAttention (QKV proj + MHA + out-proj + residual + LayerNorm) on 8
Trainium2 NeuronCores.

Sharding: tensor-parallel over heads. Core c owns heads {2c, 2c+1} (a
128-wide slice of the hidden dim): it computes Q/K/V projections for its
slice over the full batch*seq, runs attention for its 8 (batch, head)
pairs entirely out of SBUF. A per-batch AllToAll re-shards the attention
context from head-split to row-split: core c owns rows
[b*2048 + c*256, b*2048 + (c+1)*256) of EVERY batch b, so the output
projection + residual + LayerNorm for batch b overlaps the attention
compute of batch b+1 (instead of one serial tail after a single A2A).
Host reassembles the 8 cores' [B, 256, H] row shards.

Matmuls run in bf16 (fp32 PSUM accumulate); softmax and LayerNorm
arithmetic stay fp32.

softmax: scores are built transposed (scoresT[k, q] per head) so the
probs@V contraction needs no transpose; the row-sum comes from an extra
all-ones column appended to V; the attention mask enters as the
per-partition bias of the Exp activation (exp(s/8 + mask_k), mask only
depends on the key position k which is the partition axis of scoresT).

LayerNorm avoids ACT-table switches (Exp must stay resident for the
interleaved attention): mean/var via bn_stats/bn_aggr on the vector
engine, rstd via a Quake-style bit-trick + 2 Newton iterations on the
vector engine (the hardware has no rsqrt outside the ACT tables).

Scheduling notes (the bulk of the speedup over the first working
version): one big host-preswizzled DMA descriptor per tensor (the sync
engine pays ~0.7us per trigger); compute-dependent DMA triggers (A2A
staging, output stores) issue from the gpsimd queue so they never block
the sync queue's input stream; a tiny memset "fence" on the ctxF tile
keeps the collective-dependent out-proj matmuls from being statically
scheduled ahead of ready attention work; deep hsb buffering (bufs=7)
decouples next-batch input loads from the V-projection JIT scheduling;
batch 0 interleaves attention qg0 into the projection blocks; throwaway
matmuls warm the PE clock (HAM) through the startup DMA wait and the
final collective.
"""

import os
import sys
import contextlib
import ctypes
import types

import numpy as np
import ml_dtypes

N_CORES = 8
B, S, H = 4, 2048, 1024
NH, DH = 16, 64
R = B * S            # 8192 flattened rows
HB = H // N_CORES    # 128 head-dim columns per core (2 heads)
RPB = S // N_CORES   # 256 output rows per core per batch
SBW = 512            # seq-block width for projections
NSB = R // SBW       # 16 seq blocks
NHC = H // 128       # 8 contraction chunks of 128
NKB = S // 128       # 16 key blocks per batch
LN_EPS = 1e-12

last_exec_time_ns = None

# ---------------------------------------------------------------------------
# NTFF profile hook shim (axon images without antenv.axon_hooks).
# Only needed when tracing; harmless otherwise.
_SO_PATH = "/opt/axon/libaxon_pjrt.so"


def _install_ntff_shim():
    try:
        from antenv import axon_hooks  # noqa: F401
        return
    except ImportError:
        pass
    hook = None
    try:
        lib = ctypes.CDLL(_SO_PATH)
        if hasattr(lib, "axon_start_nrt_profile"):
            lib.axon_start_nrt_profile.argtypes = [
                ctypes.POINTER(ctypes.c_int64), ctypes.c_size_t]
            lib.axon_start_nrt_profile.restype = ctypes.c_int64
            lib.axon_stop_nrt_profile.argtypes = [ctypes.c_char_p]
            lib.axon_stop_nrt_profile.restype = ctypes.c_int64

            @contextlib.contextmanager
            def _hook(output_dir, device_ids):
                import jax
                jax.devices()
                if device_ids:
                    ids = (ctypes.c_int64 * len(device_ids))(*device_ids)
                    rc = lib.axon_start_nrt_profile(ids, len(device_ids))
                else:
                    rc = lib.axon_start_nrt_profile(None, 0)
                if rc != 0:
                    raise RuntimeError(f"axon_start_nrt_profile rc={rc}")
                try:
                    yield
                finally:
                    n = lib.axon_stop_nrt_profile(str(output_dir).encode())
                    print(f"profile: {n} ntff file(s) in {output_dir}",
                          file=sys.stderr)

            hook = _hook
    except OSError:
        pass
    mod = types.ModuleType("antenv.axon_hooks")
    mod._hook = hook
    mod.get_axon_ntff_profile_hook = lambda: mod._hook
    mod.set_axon_ntff_profile_hook = lambda h: setattr(mod, "_hook", h)
    sys.modules["antenv.axon_hooks"] = mod
    try:
        import antenv
        antenv.axon_hooks = mod
    except ImportError:
        pass


# ---------------------------------------------------------------------------

def _build():
    from concourse import bacc, tile
    import concourse.mybir as mybir

    f32 = mybir.dt.float32
    bf16 = mybir.dt.bfloat16
    AF = mybir.ActivationFunctionType
    ALU = mybir.AluOpType

    nc = bacc.Bacc("TRN2", target_bir_lowering=False, debug=False,
                   num_devices=N_CORES)

    # ---- DRAM I/O ----
    # Multi-chunk tensors are pre-swizzled on the host so each SBUF load is
    # ONE big DMA descriptor (the sync engine pays ~0.7us per trigger).
    hT_d = nc.dram_tensor("hT", [128, NSB, NHC, SBW], bf16,
                          kind="ExternalInput")
    wqT_d = nc.dram_tensor("wqT", [128, NHC, HB], bf16, kind="ExternalInput")
    wkT_d = nc.dram_tensor("wkT", [128, NHC, HB], bf16, kind="ExternalInput")
    wvT_d = nc.dram_tensor("wvT", [128, NHC, HB], bf16, kind="ExternalInput")
    woT_d = nc.dram_tensor("woT", [128, NHC, H], bf16, kind="ExternalInput")
    bq_d = nc.dram_tensor("bq", [HB], f32, kind="ExternalInput")
    bk_d = nc.dram_tensor("bk", [HB], f32, kind="ExternalInput")
    bv_d = nc.dram_tensor("bv", [HB], f32, kind="ExternalInput")
    bo_d = nc.dram_tensor("bo", [H], f32, kind="ExternalInput")
    gamma_d = nc.dram_tensor("gamma", [H], f32, kind="ExternalInput")
    beta_d = nc.dram_tensor("beta", [H], f32, kind="ExternalInput")
    maskT_d = nc.dram_tensor("maskT", [128, NKB, B], f32,
                             kind="ExternalInput")
    hres_d = nc.dram_tensor("hres", [B, 128, RPB // 128, H], f32,
                            kind="ExternalInput")
    out_d = nc.dram_tensor("out", [B, RPB, H], f32, kind="ExternalOutput")

    with tile.TileContext(nc) as tc:
        with (
            tc.tile_pool(name="const", bufs=1) as cpool,
            tc.tile_pool(name="psA", bufs=2, space="PSUM") as psA,
            tc.tile_pool(name="psB", bufs=2, space="PSUM") as psB,
            tc.tile_pool(name="psC", bufs=2, space="PSUM") as psC,
            tc.tile_pool(name="dram", bufs=1, space="DRAM") as dpool,
            tc.tile_pool(name="attn", bufs=2) as apool,
            tc.tile_pool(name="ptp", bufs=4) as ptpool,
            tc.tile_pool(name="bp", bufs=3) as bpool,
            tc.tile_pool(name="outp", bufs=2) as opool,
        ):
            # ================= setup (projection-phase constants) =========
            # K/Q weights first (K-projections lead), then the first hsb
            # blocks; the small bias/mask/wv loads are emitted after
            # emit_proj(0) so they queue behind the first hidden blocks.
            wq_sb = cpool.tile([128, NHC, HB], bf16, tag="wq")
            wk_sb = cpool.tile([128, NHC, HB], bf16, tag="wk")
            wv_sb = cpool.tile([128, NHC, HB], bf16, tag="wv")
            nc.sync.dma_start(wk_sb[:, :, :], wkT_d[:, :, :])
            nc.sync.dma_start(wq_sb[:, :, :], wqT_d[:, :, :])

            # PE warm-up: throwaway matmuls on the K weights keep the
            # HAM activity window busy while the first hidden blocks stream
            # in, so the first projections run at 2.4 GHz instead of 1.2.
            for w in range(16):
                pwarm = psA.tile([128, SBW], f32, tag="proj")
                nc.tensor.matmul(pwarm[:, :], wk_sb[:, w % NHC, :],
                                 wk_sb[:, 0:4, :], start=True, stop=True)

            bq_sb = cpool.tile([128, 1], f32, tag="bq")
            bk_sb = cpool.tile([128, 1], f32, tag="bk")
            bv_b = cpool.tile([128, HB], f32, tag="bv_b")
            mask_sb = cpool.tile([128, NKB, B], f32, tag="mask")

            def emit_small_loads():
                nc.sync.dma_start(wv_sb[:, :, :], wvT_d[:, :, :])
                nc.sync.dma_start(bk_sb[:, :], bk_d[:].unsqueeze(1))
                nc.sync.dma_start(bq_sb[:, :], bq_d[:].unsqueeze(1))
                # bv broadcast along partitions (V is in [seq, d] layout)
                nc.sync.dma_start(
                    bv_b[:, :],
                    bv_d[:].unsqueeze(0).partition_broadcast(128))
                # raw mask [k-in-block, kblock, batch]; consumed as the
                # per-partition bias of the Exp activation.
                nc.sync.dma_start(mask_sb[:, :, :], maskT_d[:, :, :])

            # A2A staging buffers (DRAM), one pair per batch
            a2a_in = [dpool.tile([N_CORES, 128, RPB], bf16, tag=f"a2a_in{b}",
                                 name=f"a2a_in{b}")
                      for b in range(B)]
            a2a_out = [dpool.tile([N_CORES, 128, RPB], bf16,
                                  tag=f"a2a_out{b}", name=f"a2a_out{b}")
                       for b in range(B)]

            # out-proj phase constants: declared here, loaded (by emission
            # order => scheduler priority) after batch 0 is enqueued so the
            # big wo/hres DMAs don't delay the first hidden-state blocks.
            wo_sb = cpool.tile([128, NHC, H], bf16, tag="wo")
            bo_b = cpool.tile([128, H], f32, tag="bo_b")
            gamma_b = cpool.tile([128, H], f32, tag="gamma_b")
            beta_b = cpool.tile([128, H], f32, tag="beta_b")


            def emit_kb(b, qg, kb, qt_sb, kt_sb, va, pc0, pc1):
                """One kb step of attention: scores pair (row-tile packed),
                exp (mask as ACT bias), probs@V accumulate."""
                sblk = kb // 4
                kcol = 128 * (kb % 4)
                sc = psB.tile([128, 2 * SBW], f32, tag="sc")
                nc.tensor.matmul(
                    sc[:, 0:SBW],
                    kt_sb[0:DH, sblk, kcol:kcol + 128],
                    qt_sb[0:DH, qg, :],
                    start=True, stop=True)
                nc.tensor.matmul(
                    sc[:, SBW:2 * SBW],
                    kt_sb[DH:2 * DH, sblk, kcol:kcol + 128],
                    qt_sb[DH:2 * DH, qg, :],
                    start=True, stop=True)
                pt = ptpool.tile([128, 2 * SBW], bf16, tag="pt")
                nc.scalar.activation(pt[:, :], sc[:, :], AF.Exp,
                                     bias=mask_sb[:, kb, b].unsqueeze(1),
                                     scale=0.125)
                nc.tensor.matmul(pc0[:, :], va[0][:, kb, :],
                                 pt[:, 0:SBW],
                                 start=(kb == 0), stop=(kb == NKB - 1))
                nc.tensor.matmul(pc1[:, :], va[1][:, kb, :],
                                 pt[:, SBW:2 * SBW],
                                 start=(kb == 0), stop=(kb == NKB - 1))

            def emit_normalize(b, qg, ctxT_b, pc0, pc1):
                """ctxT[d, q] = ctx'[d, q] / rowsum[q]. Copies release the
                psC slots first; reciprocal runs broadcast (64 lanes)."""
                cu = apool.tile([DH, 2, SBW], f32, tag="cu")
                rs = apool.tile([1, 2, SBW], f32, tag="rs")
                nc.vector.tensor_copy(cu[:, 0, :], pc0[0:DH, :])
                nc.vector.tensor_copy(rs[:, 0, :], pc0[DH:DH + 1, :])
                nc.vector.tensor_copy(cu[:, 1, :], pc1[0:DH, :])
                nc.vector.tensor_copy(rs[:, 1, :], pc1[DH:DH + 1, :])
                rb = [apool.tile([DH, SBW], f32, tag=f"rb{h}",
                                 name=f"rb{h}") for h in range(2)]
                for h in range(2):
                    nc.gpsimd.partition_broadcast(rb[h][:, :], rs[:, h, :])
                    nc.vector.reciprocal_approx_fast(rb[h][:, :],
                                                     rb[h][:, :])
                nc.vector.tensor_mul(ctxT_b[0:DH, qg, :],
                                     cu[0:DH, 0, :], rb[0][:, :])
                nc.vector.tensor_mul(ctxT_b[DH:2 * DH, qg, :],
                                     cu[0:DH, 1, :], rb[1][:, :])

            def emit_batch(b, va, qt_sb, kt_sb, ctxT_b, mid_emit):
                """Projections for batch b interleaved with attention qg0
                (qg0's kb 4j..4j+3 only need key/value block j), then Q for
                blocks 1-3, then attention qg1-3. For b>0 the projections
                get a priority offset so the static schedule interleaves
                them into attention(b-1)'s PE gaps instead of serializing
                them at the batch boundary."""
                import contextlib
                hsbs = []
                for i in range(4 * b, 4 * b + 4):
                    hsb = apool.tile([128, NHC, SBW], bf16, tag="hsb",
                                     bufs=8)
                    nc.sync.dma_start(hsb[:, :, :], hT_d[:, i, :, :])
                    hsbs.append(hsb)
                if b == 0:
                    emit_small_loads()
                pc0_0 = psC.tile([DH + 1, SBW], f32, tag="ctx")
                pc1_0 = psC.tile([DH + 1, SBW], f32, tag="ctx")
                for j in range(4):
                    i = 4 * b + j
                    hsb = hsbs[j]
                    # K^T block
                    pk = psA.tile([128, SBW], f32, tag="proj")
                    for c in range(NHC):
                        nc.tensor.matmul(pk[:, :], wk_sb[:, c, :],
                                         hsb[:, c, :],
                                         start=(c == 0),
                                         stop=(c == NHC - 1))
                    nc.vector.tensor_scalar_add(kt_sb[:, j, :],
                                                pk[:, :], bk_sb[:, :])
                    if j == 0 or b > 0:
                        # Q^T block
                        pq = psA.tile([128, SBW], f32, tag="proj")
                        for c in range(NHC):
                            nc.tensor.matmul(pq[:, :], wq_sb[:, c, :],
                                             hsb[:, c, :],
                                             start=(c == 0),
                                             stop=(c == NHC - 1))
                        nc.vector.tensor_scalar_add(qt_sb[:, j, :],
                                                    pq[:, :], bq_sb[:, :])
                    # V in natural [seq, d] layout, 4 sub-blocks of 128
                    for sub in range(4):
                        kb = 4 * j + sub
                        pv = psA.tile([128, SBW], f32, tag="proj")
                        for c in range(NHC):
                            nc.tensor.matmul(
                                pv[:, 0:HB],
                                hsb[:, c, 128 * sub:128 * (sub + 1)],
                                wv_sb[:, c, :],
                                start=(c == 0), stop=(c == NHC - 1))
                        nc.vector.tensor_add(va[0][:, kb, 0:DH],
                                             pv[:, 0:DH], bv_b[:, 0:DH])
                        nc.vector.tensor_add(va[1][:, kb, 0:DH],
                                             pv[:, DH:HB], bv_b[:, DH:HB])
                    if b == 0:
                        # batch 0 only: start qg0 right behind each block
                        # (for b>0 this would park proj(b) behind an exp-
                        # gated probs@V in the PE FIFO during attn(b-1)).
                        for kb in range(4 * j, 4 * j + 4):
                            emit_kb(b, 0, kb, qt_sb, kt_sb, va,
                                    pc0_0, pc1_0)
                if b == 0:
                    # remaining Q blocks
                    for j in range(1, 4):
                        hsb = hsbs[j]
                        pq = psA.tile([128, SBW], f32, tag="proj")
                        for c in range(NHC):
                            nc.tensor.matmul(pq[:, :], wq_sb[:, c, :],
                                             hsb[:, c, :],
                                             start=(c == 0),
                                             stop=(c == NHC - 1))
                        nc.vector.tensor_scalar_add(qt_sb[:, j, :],
                                                    pq[:, :], bq_sb[:, :])
                else:
                    for kb in range(NKB):
                        emit_kb(b, 0, kb, qt_sb, kt_sb, va, pc0_0, pc1_0)
                emit_normalize(b, 0, ctxT_b, pc0_0, pc1_0)
                if mid_emit is not None:
                    mid_emit()
                for qg in range(1, 4):
                    pc0 = psC.tile([DH + 1, SBW], f32, tag="ctx")
                    pc1 = psC.tile([DH + 1, SBW], f32, tag="ctx")
                    for kb in range(NKB):
                        emit_kb(b, qg, kb, qt_sb, kt_sb, va, pc0, pc1)
                    emit_normalize(b, qg, ctxT_b, pc0, pc1)

            i32 = mybir.dt.int32
            # Schraudolph/Quake constant: 1.5 * (127 - 0.0450466) * 2^23
            QK_C = 1.5 * (127.0 - 0.0450466) * 8388608.0

            def emit_outproj(b, ctxFb, hres_b):
                """Out-proj + residual + LayerNorm for this core's 256 rows
                of batch b. No ACT-table switches: LN runs on DVE only,
                rstd via bit-trick + 2 Newton iterations (rel err ~5e-6)."""
                nt = RPB // 128
                for t in range(nt):
                    nc.vector.tensor_add(hres_b[:, t, :], hres_b[:, t, :],
                                         bo_b[:, :])
                mv_b = opool.tile([128, nt, 2], f32, tag="bnmv")
                xs = []
                for t in range(nt):
                    x_sb = opool.tile([128, H], f32, tag=f"xln{t}",
                                      name=f"xln{t}", bufs=1)
                    for g in range(2):
                        po = psA.tile([128, SBW], f32, tag="proj")
                        for c in range(NHC):
                            nc.tensor.matmul(
                                po[:, :],
                                ctxFb[:, c, 128 * t:128 * (t + 1)],
                                wo_sb[:, c, SBW * g:SBW * (g + 1)],
                                start=(c == 0), stop=(c == NHC - 1))
                        nc.vector.tensor_add(
                            x_sb[:, SBW * g:SBW * (g + 1)], po[:, :],
                            hres_b[:, t, SBW * g:SBW * (g + 1)])
                    stats = opool.tile([128, 2, 6], f32, tag="bnst")
                    nc.vector.bn_stats(stats[:, 0, :], x_sb[:, 0:SBW])
                    nc.vector.bn_stats(stats[:, 1, :], x_sb[:, SBW:2 * SBW])
                    nc.vector.bn_aggr(mv_b[:, t, :], stats[:, :, :])
                    xs.append(x_sb)
                # rstd = (var + eps)^-0.5 for all row-tiles at once [128, nt]
                vv = opool.tile([128, nt], f32, tag="q_vv")
                nc.vector.tensor_scalar_add(vv[:, :], mv_b[:, :, 1], LN_EPS)
                qf = opool.tile([128, nt], f32, tag="q_f")
                nc.vector.tensor_copy(qf[:, :], vv[:, :].bitcast(i32))
                nc.vector.tensor_scalar(qf[:, :], qf[:, :], -0.5, QK_C,
                                        ALU.mult, ALU.add)
                qi = opool.tile([128, nt], i32, tag="q_i")
                nc.vector.tensor_copy(qi[:, :], qf[:, :])
                y0 = qi[:, :].bitcast(f32)
                rstd = opool.tile([128, nt], f32, tag="rstd")
                tn = opool.tile([128, nt], f32, tag="q_tn")
                for it in range(2):
                    src = y0 if it == 0 else rstd[:, :]
                    nc.vector.tensor_mul(tn[:, :], vv[:, :], src)
                    nc.vector.tensor_mul(tn[:, :], tn[:, :], src)
                    nc.vector.tensor_scalar(tn[:, :], tn[:, :], -0.5, 1.5,
                                            ALU.mult, ALU.add)
                    nc.vector.tensor_mul(rstd[:, :], src, tn[:, :])
                for t in range(nt):
                    # y = ((x - mu) * gamma) * rstd + beta
                    y_sb = opool.tile([128, H], f32, tag="yln")
                    nc.vector.scalar_tensor_tensor(y_sb[:, :], xs[t][:, :],
                                                   mv_b[:, t, 0:1],
                                                   gamma_b[:, :],
                                                   ALU.subtract, ALU.mult)
                    nc.vector.tensor_scalar_mul(y_sb[:, :], y_sb[:, :],
                                                rstd[:, t:t + 1])
                    nc.vector.tensor_add(y_sb[:, :], y_sb[:, :],
                                         beta_b[:, :])
                    nc.gpsimd.dma_start(out_d[b, 128 * t:128 * (t + 1), :],
                                        y_sb[:, :])

            # ================= main pipeline ==============================
            hres_tiles = {}
            for b in range(B):
                va = [apool.tile([128, NKB, DH + 1], bf16, tag=f"va{h}",
                                 name=f"va{h}") for h in range(2)]
                for h in range(2):
                    nc.vector.memset(va[h][:, :, DH:DH + 1], 1.0)
                qt_sb = apool.tile([128, 4, SBW], bf16, tag="qt")
                kt_sb = apool.tile([128, 4, SBW], bf16, tag="kt")
                ctxT_b = bpool.tile([128, 4, SBW], bf16, tag="ctxT")

                def mid_emit(b=b):
                    """Emitted right after qg0(b)'s normalize: out-proj of
                    batch b-1. The tiny memset is a scheduling fence -- it
                    makes the cc-dependent out-proj matmuls depend on
                    in-pipeline work, so the static PE order can't park them
                    (and everything behind them) on the collective."""
                    if b == 0:
                        return
                    ctxFb = opool.tile([128, N_CORES, RPB], bf16,
                                       tag="ctxF", name="ctxFb")
                    nc.vector.memset(ctxFb[0:1, 0:1], 0.0)
                    for src_ in range(N_CORES):
                        nc.sync.dma_start(ctxFb[:, src_, :],
                                          a2a_out[b - 1][src_, :, :])
                    emit_outproj(b - 1, ctxFb, hres_tiles.pop(b - 1))

                emit_batch(b, va, qt_sb, kt_sb, ctxT_b, mid_emit)
                # stage this batch's 8 row-chunks and run its AllToAll.
                # Triggered from the gpsimd queue (not sync, where it would
                # block the input-stream DMAs of the next batch).
                for j in range(N_CORES):
                    eng = nc.scalar if (b == B - 1 and j % 2 == 1) else nc.gpsimd
                    eng.dma_start(
                        a2a_in[b][j, :, :],
                        ctxT_b[:, j // 2, RPB * (j % 2):RPB * (j % 2 + 1)])
                nc.gpsimd.collective_compute(
                    "AllToAll", ALU.bypass,
                    replica_groups=[list(range(N_CORES))],
                    ins=[a2a_in[b][:].opt()], outs=[a2a_out[b][:].opt()])

                if b == 0:
                    # deprioritized big loads: needed first by outproj(0),
                    # which runs during batch 1's attention.
                    nc.sync.dma_start(wo_sb[:, :, :], woT_d[:, :, :])
                    nc.sync.dma_start(
                        bo_b[:, :],
                        bo_d[:].unsqueeze(0).partition_broadcast(128))
                    nc.sync.dma_start(
                        gamma_b[:, :],
                        gamma_d[:].unsqueeze(0).partition_broadcast(128))
                    nc.sync.dma_start(
                        beta_b[:, :],
                        beta_d[:].unsqueeze(0).partition_broadcast(128))

                # residual rows for batch b, consumed by outproj(b)
                hres_b = opool.tile([128, RPB // 128, H], f32, tag="hres")
                nc.sync.dma_start(hres_b[:, :, :], hres_d[b, :, :, :])
                hres_tiles[b] = hres_b

            # keep the PE's HAM activity window busy through the last
            # collective so outproj(3) runs at full clock, not 1.2 GHz
            for w in range(48):
                pwarm = psA.tile([128, SBW], f32, tag="proj")
                nc.tensor.matmul(pwarm[:, :], wo_sb[:, w % NHC, 0:128],
                                 wo_sb[:, w % NHC, 0:SBW],
                                 start=True, stop=True)
            ctxFb = opool.tile([128, N_CORES, RPB], bf16, tag="ctxF",
                               name="ctxFb")
            for src_ in range(N_CORES):
                eng = nc.scalar if src_ % 2 == 1 else nc.sync
                eng.dma_start(ctxFb[:, src_, :],
                              a2a_out[B - 1][src_, :, :])
            emit_outproj(B - 1, ctxFb, hres_tiles.pop(B - 1))

    nc.compile()
    return nc


_NC_CACHE = None


def _get_nc():
    global _NC_CACHE
    if _NC_CACHE is None:
        _NC_CACHE = _build()
    return _NC_CACHE


def _swz(mat_T):
    """[128*C, F] -> [128, C, F] with partition dim first (one-DMA layout)."""
    n = mat_T.shape[0] // 128
    return np.ascontiguousarray(
        mat_T.reshape(n, 128, -1).transpose(1, 0, 2))


def _make_in_maps(hidden_states, attention_mask, Wq, bq, Wk, bk, Wv, bv, Wo,
                  bo, ln_gamma, ln_beta):
    hid2 = np.asarray(hidden_states, np.float32).reshape(R, H)
    hT_bf = hid2.T.astype(ml_dtypes.bfloat16)          # [H, R]
    # [128, NSB, NHC, SBW]: hT3[p, i, c, q] = hT[128c+p, 512i+q]
    hT3 = np.ascontiguousarray(
        hT_bf.reshape(NHC, 128, NSB, SBW).transpose(1, 2, 0, 3))
    woT = _swz(np.asarray(Wo, np.float32).T.astype(ml_dtypes.bfloat16))
    maskT = _swz(np.ascontiguousarray(
        np.asarray(attention_mask, np.float32).reshape(B, S).T))
    bo32 = np.asarray(bo, np.float32)
    gamma32 = np.asarray(ln_gamma, np.float32)
    beta32 = np.asarray(ln_beta, np.float32)
    # [B, 128, RPB//128, H]: hres[b, p, t, :] = row b*S + c*RPB + 128t + p
    hid5 = hid2.reshape(B, N_CORES, RPB // 128, 128, H).transpose(0, 1, 3, 2, 4)

    in_maps = []
    for c in range(N_CORES):
        sl = slice(HB * c, HB * (c + 1))
        in_maps.append({
            "hT": hT3,
            "wqT": _swz(np.asarray(Wq, np.float32)[sl, :].T
                        .astype(ml_dtypes.bfloat16)),
            "wkT": _swz(np.asarray(Wk, np.float32)[sl, :].T
                        .astype(ml_dtypes.bfloat16)),
            "wvT": _swz(np.asarray(Wv, np.float32)[sl, :].T
                        .astype(ml_dtypes.bfloat16)),
            "woT": woT,
            "bq": np.ascontiguousarray(np.asarray(bq, np.float32)[sl]),
            "bk": np.ascontiguousarray(np.asarray(bk, np.float32)[sl]),
            "bv": np.ascontiguousarray(np.asarray(bv, np.float32)[sl]),
            "bo": bo32,
            "gamma": gamma32,
            "beta": beta32,
            "maskT": maskT,
            "hres": np.ascontiguousarray(hid5[:, c]),
        })
    return in_maps


def kernel(hidden_states, attention_mask, Wq, bq, Wk, bk, Wv, bv, Wo, bo,
           ln_gamma, ln_beta):
    global last_exec_time_ns
    from concourse.bass_utils import run_bass_kernel_spmd

    _install_ntff_shim()
    in_maps = _make_in_maps(hidden_states, attention_mask, Wq, bq, Wk, bk,
                            Wv, bv, Wo, bo, ln_gamma, ln_beta)
    nc = _get_nc()
    trace = os.environ.get("BASS_KERNEL_TRACE", "0") == "1"
    res = run_bass_kernel_spmd(nc, in_maps, core_ids=list(range(N_CORES)),
                               trace=trace)
    last_exec_time_ns = res.exec_time_ns
    if trace and res.exec_time_ns is not None:
        print(f"HW exec time: {res.exec_time_ns} ns")

    # res[c]["out"] is [B, RPB, H]; rows of batch b on core c are
    # [b*S + c*RPB, b*S + (c+1)*RPB).
    shards = np.stack([res.results[c]["out"] for c in range(N_CORES)],
                      axis=1)          # [B, N_CORES, RPB, H]
    return shards.reshape(B, S, H).astype(np.float32)
